# revision 5
# baseline (speedup 1.0000x reference)
"""Trainium2 Bass kernel for masked ALiBi attention (returns out AND p_attn).

Problem: B=2, H=16, S=2048, D=64.
  scores = QK^T/8 + alibi*slope_h, masked (mask==0 -> -1e9), p = softmax,
  out = p @ V.  Returns (out [B,H,S,D] f32, p [B,H,S,S] f32).

Sharding (8 cores, no collectives): core c -> b = c>>2, head-half hh=(c>>1)&1,
query-half qh=c&1.  Each core: 8 heads x 1024 q-rows x full 2048 k.
This minimizes HBM traffic (92 MiB/core) while amortizing per-head transposes
and per-qtile mask/alibi preprocessing.

Per-core algorithm (per head h, per 128-row q-tile, split in 2 halves of
1024 k for PSUM double-buffering):
  QT = (Q * 0.125/slope_h)^T          (PE transposes, scaled on PSUM->SBUF copy)
  KT = K^T                            (PE transposes, once per head)
  combined = alibi + 30000*mask       (DVE, once per q-tile, shared by 8 heads)
  scores = QT^T@KT (fp32r) ; scores += combined (DVE)
  p_bf16 = exp(slope*scores - 30000*slope)   (ACT, accum_out -> rowsum)
  PT = p^T (PE bf16 transposes -> PSUM -> SBUF), outT = sum_k V^T-chunks@PT
  p_out = p_bf16 * (1/rowsum) -> DMA ; out = (outT)^T * (1/rowsum) -> DMA
"""

import os
import sys

sys.path.insert(0, "/opt/trn_rl_repo")

import numpy as np

import concourse.bass as bass
import concourse.mybir as mybir
import concourse.tile as tile
from concourse import bacc
from concourse.bass_utils import run_bass_kernel_spmd
from concourse.masks import make_identity

B, H, S, D = 2, 16, 2048, 64
NCORES = 8
HLOC = 8          # heads per core
QLOC = 1024       # q rows per core
NQT = QLOC // 128  # 8 q-tiles per core
NKC = S // 128     # 16 k-chunks
MASK_BIG = 30000.0

f32 = mybir.dt.float32
f32r = mybir.dt.float32r
bf16 = mybir.dt.bfloat16
i32 = mybir.dt.int32
EXP = mybir.ActivationFunctionType.Exp
MULT = mybir.AluOpType.mult
ADD = mybir.AluOpType.add

ADD_ON_PE = os.environ.get("BASS_ADD_ON_PE", "0") == "1"


def _slopes():
    start = 2.0 ** (-8.0 / H)
    return start ** np.arange(1, H + 1, dtype=np.float64)


def build_kernel(tc, aps, hloc=HLOC, nqt=NQT):
    nc = tc.nc
    q_d, k_d, v_d, m_d, a_d, qs_d, es_d, eb_d, p_d, o_d = aps
    from contextlib import ExitStack

    ctx = ExitStack()
    with ctx:
        const = ctx.enter_context(tc.tile_pool(name="const", bufs=1))
        comb_pool = ctx.enter_context(tc.tile_pool(name="comb", bufs=1))
        kt_pool = ctx.enter_context(tc.tile_pool(name="kt", bufs=2))
        qt_pool = ctx.enter_context(tc.tile_pool(name="qt", bufs=2))
        ld_pool = ctx.enter_context(tc.tile_pool(name="ld", bufs=2))
        qld_pool = ctx.enter_context(tc.tile_pool(name="qld", bufs=3))
        work = ctx.enter_context(tc.tile_pool(name="work", bufs=2))
        pout_pool = ctx.enter_context(tc.tile_pool(name="pout", bufs=3))
        small = ctx.enter_context(tc.tile_pool(name="small", bufs=6))
        psum = ctx.enter_context(tc.tile_pool(name="psum", bufs=2, space="PSUM"))
        psumx = ctx.enter_context(tc.tile_pool(name="psumx", bufs=4, space="PSUM"))

        # constants
        ident_f32 = const.tile([128, 128], f32)
        make_identity(nc, ident_f32[:])
        ident_bf16 = const.tile([128, 128], bf16)
        nc.vector.tensor_copy(ident_bf16[:], ident_f32[:])
        qs_sb = const.tile([64, hloc], f32)
        nc.sync.dma_start(qs_sb[:], qs_d[:])
        es_sb = const.tile([128, hloc], f32)
        nc.sync.dma_start(es_sb[:], es_d[:])
        eb_sb = const.tile([128, hloc], f32)
        nc.sync.dma_start(eb_sb[:], eb_d[:])

        # combined[qt] = alibi + 30000*mask  (f32, resident, shared by all heads)
        comb = comb_pool.tile([128, nqt * 2048], f32)
        for qt in range(nqt):
            cs = comb[:, qt * 2048:(qt + 1) * 2048]
            nc.sync.dma_start(cs, a_d[qt * 128:(qt + 1) * 128, :])
            m_st = ld_pool.tile([128, 2048], i32, tag="mst")
            nc.sync.dma_start(m_st[:], m_d[qt * 128:(qt + 1) * 128, :])
            nc.vector.scalar_tensor_tensor(
                out=cs, in0=m_st[:], scalar=MASK_BIG, in1=cs, op0=MULT, op1=ADD
            )

        for h in range(hloc):
            # ---- per-head setup: load K/V, build KT (f32) and QT (f32, scaled)
            k_st = ld_pool.tile([128, NKC, 64], f32, tag="kst")
            nc.sync.dma_start(
                k_st[:], k_d[h].rearrange("(c p) d -> p c d", p=128)
            )
            v_bf = ld_pool.tile([128, NKC, 64], bf16, tag="vbf")
            nc.gpsimd.dma_start(
                out=v_bf[:], in_=v_d[h].rearrange("(c p) d -> p c d", p=128)
            )
            kt = kt_pool.tile([64, S], f32r, tag="kt")
            for r in range(4):
                st = psumx.tile([64, 512], f32, tag="ptx")
                for j in range(4):
                    c = r * 4 + j
                    nc.tensor.transpose(
                        st[:, j * 128:(j + 1) * 128],
                        k_st[:, c, :],
                        ident_f32[:],
                    )
                nc.any.tensor_copy(kt[:, r * 512:(r + 1) * 512], st[:])
            qt_t = qt_pool.tile([64, nqt * 128], f32r, tag="qt")
            for r in range((nqt + 3) // 4):
                nj = min(4, nqt - r * 4)
                st = psumx.tile([64, 512], f32, tag="ptx")
                for j in range(nj):
                    qi = r * 4 + j
                    q_st = qld_pool.tile([128, 64], f32, tag="qst")
                    nc.sync.dma_start(q_st[:], q_d[h, qi * 128:(qi + 1) * 128, :])
                    nc.tensor.transpose(
                        st[:, j * 128:(j + 1) * 128], q_st[:], ident_f32[:]
                    )
                nc.vector.tensor_scalar(
                    out=qt_t[:, r * 512:r * 512 + nj * 128],
                    in0=st[:, : nj * 128],
                    scalar1=qs_sb[:, h:h + 1],
                    scalar2=None,
                    op0=MULT,
                )

            for qi in range(nqt):
                # ---- main tile: 128 q-rows x 2048 k, processed in 2 halves
                lhsT = qt_t[:, qi * 128:(qi + 1) * 128]
                p_bf = work.tile([128, S], bf16, tag="pbf")
                pt_sb = work.tile([128, S], bf16, tag="ptsb")
                p_out = pout_pool.tile([128, S], f32, tag="pout")
                outT = psumx.tile([64, 128], f32, tag="ptx")
                rs = []
                for half in range(2):
                    sc = psum.tile([128, 1024], f32, tag="scores")
                    for j in range(2):
                        kcol = half * 1024 + j * 512
                        nc.tensor.matmul(
                            sc[:, j * 512:(j + 1) * 512],
                            lhsT=lhsT,
                            rhs=kt[:, kcol:kcol + 512],
                            start=True,
                            stop=not ADD_ON_PE,
                        )
                        if ADD_ON_PE:
                            nc.tensor.matmul(
                                sc[:, j * 512:(j + 1) * 512],
                                lhsT=ident_f32.bitcast(f32r),
                                rhs=comb[
                                    :, qi * 2048 + kcol:qi * 2048 + kcol + 512
                                ].bitcast(f32r),
                                start=False,
                                stop=True,
                            )
                    if not ADD_ON_PE:
                        cslice = comb[
                            :, qi * 2048 + half * 1024:qi * 2048 + half * 1024 + 1024
                        ]
                        nc.vector.tensor_add(sc[:], sc[:], cslice)
                    rs_h = small.tile([128, 1], f32, tag="rs")
                    nc.scalar.activation(
                        p_bf[:, half * 1024:(half + 1) * 1024],
                        sc[:],
                        EXP,
                        bias=eb_sb[:, h:h + 1],
                        scale=es_sb[:, h:h + 1],
                        accum_out=rs_h[:],
                    )
                    rs.append(rs_h)
                    # transpose p (bf16) for the PV matmul
                    pt_ps = psumx.tile([128, 1024], bf16, tag="ptx")
                    for c in range(8):
                        col = half * 1024 + c * 128
                        nc.tensor.transpose(
                            pt_ps[:, c * 128:(c + 1) * 128],
                            p_bf[:, col:col + 128],
                            ident_bf16[:],
                        )
                    nc.any.tensor_copy(
                        pt_sb[:, half * 1024:(half + 1) * 1024], pt_ps[:]
                    )
                    for c in range(8):
                        cg = half * 8 + c
                        nc.tensor.matmul(
                            outT[:],
                            lhsT=v_bf[:, cg, :],
                            rhs=pt_sb[:, cg * 128:(cg + 1) * 128],
                            start=(cg == 0),
                            stop=(cg == NKC - 1),
                        )
                # rowsum -> 1/sum ; normalize p ; fix up out
                rsum = small.tile([128, 1], f32, tag="rs")
                nc.vector.tensor_add(rsum[:], rs[0][:], rs[1][:])
                recip = small.tile([128, 1], f32, tag="rs")
                nc.vector.reciprocal(recip[:], rsum[:])
                for half in range(2):
                    nc.vector.tensor_scalar(
                        out=p_out[:, half * 1024:(half + 1) * 1024],
                        in0=p_bf[:, half * 1024:(half + 1) * 1024],
                        scalar1=recip[:],
                        scalar2=None,
                        op0=MULT,
                    )
                outT_sb = small.tile([64, 128], f32, tag="otsb")
                nc.any.tensor_copy(outT_sb[:], outT[:])
                fix_ps = psumx.tile([128, 64], f32, tag="ptx")
                nc.tensor.transpose(
                    fix_ps[:], outT_sb[:], ident_f32[:64, :64]
                )
                out_sb = small.tile([128, 64], f32, tag="osb")
                nc.vector.tensor_scalar(
                    out=out_sb[:],
                    in0=fix_ps[:],
                    scalar1=recip[:],
                    scalar2=None,
                    op0=MULT,
                )
                nc.sync.dma_start(
                    p_d[h, qi * 128:(qi + 1) * 128, :], p_out[:]
                )
                nc.sync.dma_start(
                    o_d[h, qi * 128:(qi + 1) * 128, :], out_sb[:]
                )


def build_program(hloc=HLOC, qloc=QLOC):
    nqt = qloc // 128
    nc = bacc.Bacc(
        "TRN2", target_bir_lowering=False, debug=False, num_devices=NCORES
    )
    q_d = nc.dram_tensor("q", [hloc, qloc, D], f32, kind="ExternalInput").ap()
    k_d = nc.dram_tensor("k", [hloc, S, D], f32, kind="ExternalInput").ap()
    v_d = nc.dram_tensor("v", [hloc, S, D], f32, kind="ExternalInput").ap()
    m_d = nc.dram_tensor("mask", [qloc, S], i32, kind="ExternalInput").ap()
    a_d = nc.dram_tensor("alibi", [qloc, S], f32, kind="ExternalInput").ap()
    qs_d = nc.dram_tensor("qscale", [64, hloc], f32, kind="ExternalInput").ap()
    es_d = nc.dram_tensor("escale", [128, hloc], f32, kind="ExternalInput").ap()
    eb_d = nc.dram_tensor("ebias", [128, hloc], f32, kind="ExternalInput").ap()
    p_d = nc.dram_tensor("p", [hloc, qloc, S], f32, kind="ExternalOutput").ap()
    o_d = nc.dram_tensor("o", [hloc, qloc, D], f32, kind="ExternalOutput").ap()
    aps = (q_d, k_d, v_d, m_d, a_d, qs_d, es_d, eb_d, p_d, o_d)
    with tile.TileContext(nc) as tc:
        build_kernel(tc, aps, hloc=hloc, nqt=nqt)
    nc.compile()
    return nc


_CACHE = {}


def _get_program():
    if "nc" not in _CACHE:
        _CACHE["nc"] = build_program()
    return _CACHE["nc"]


def _make_in_maps(query, key, value, mask, alibi):
    slopes = _slopes()
    in_maps = []
    for c in range(NCORES):
        b, hh, qh = c >> 2, (c >> 1) & 1, c & 1
        h0, q0 = hh * HLOC, qh * QLOC
        sl = slopes[h0:h0 + HLOC]
        in_maps.append({
            "q": np.ascontiguousarray(
                query[b, h0:h0 + HLOC, q0:q0 + QLOC, :], dtype=np.float32
            ),
            "k": np.ascontiguousarray(key[b, h0:h0 + HLOC], dtype=np.float32),
            "v": np.ascontiguousarray(value[b, h0:h0 + HLOC], dtype=np.float32),
            "mask": np.ascontiguousarray(
                mask[b, 0, q0:q0 + QLOC, :], dtype=np.int32
            ),
            "alibi": np.ascontiguousarray(
                alibi[b, q0:q0 + QLOC, :], dtype=np.float32
            ),
            "qscale": np.tile(
                (0.125 / sl).astype(np.float32)[None, :], (64, 1)
            ),
            "escale": np.tile(sl.astype(np.float32)[None, :], (128, 1)),
            "ebias": np.tile(
                (-MASK_BIG * sl).astype(np.float32)[None, :], (128, 1)
            ),
        })
    return in_maps


def _gather(results):
    out = np.zeros((B, H, S, D), np.float32)
    p = np.zeros((B, H, S, S), np.float32)
    for c in range(NCORES):
        b, hh, qh = c >> 2, (c >> 1) & 1, c & 1
        h0, q0 = hh * HLOC, qh * QLOC
        out[b, h0:h0 + HLOC, q0:q0 + QLOC, :] = results[c]["o"]
        p[b, h0:h0 + HLOC, q0:q0 + QLOC, :] = results[c]["p"]
    return out, p


def kernel(query, key, value, mask, alibi):
    nc = _get_program()
    in_maps = _make_in_maps(query, key, value, mask, alibi)
    res = run_bass_kernel_spmd(nc, in_maps, core_ids=list(range(NCORES)))
    return _gather(res.results)


def _ensure_ntff_hook():
    """Wire the axon NTFF profile hook into the stub antenv package."""
    import types

    try:
        import antenv.axon_hooks  # noqa: F401

        return
    except ImportError:
        pass
    import antenv
    from trn_agent_boot.trn_boot import _ntff_profile_via_ctypes

    hook = _ntff_profile_via_ctypes("/opt/axon/libaxon_pjrt.so")
    mod = types.ModuleType("antenv.axon_hooks")
    mod.get_axon_ntff_profile_hook = lambda: hook
    mod.set_axon_ntff_profile_hook = lambda h: None
    sys.modules["antenv.axon_hooks"] = mod
    antenv.axon_hooks = mod

    import concourse.bass_utils as bu

    if not getattr(bu, "_upload_patched", False):
        orig = bu.upload_artifacts

        def _safe_upload(tmpdir):
            try:
                return orig(tmpdir)
            except Exception as e:  # no artifact bucket in this container
                return f"upload-skipped: {e}"

        bu.upload_artifacts = _safe_upload
        bu._upload_patched = True


def kernel_traced(query, key, value, mask, alibi, **kw):
    """Like kernel(), but with NTFF profiling; returns (outputs, BassKernelResults)."""
    _ensure_ntff_hook()
    nc = _get_program()
    in_maps = _make_in_maps(query, key, value, mask, alibi)
    res = run_bass_kernel_spmd(
        nc, in_maps, core_ids=list(range(NCORES)), trace=True, **kw
    )
    return _gather(res.results), res
